# revision 28
# baseline (speedup 1.0000x reference)
"""Trainium2 Bass kernel for rank-1 attention + linear (nn_Attention).

Reference computation (S=256, B=128, D=4096):
    scores   = einsum('sbd,bd->bs', inp, hidden[0])      # dot each enc state with hidden
    attn     = softmax(scores, axis=1)                   # over S
    weighted = einsum('bs,sbd->bd', attn, inp)
    concat   = [weighted, hidden[0]]   # [B, 2D]
    out      = concat @ W.T + b        # [1, B, D]

Distribution over 8 NeuronCores:
  - attention part: data-parallel over B (16 batches per core)
  - linear part: W sharded over output dim (512 rows per core); weighted
    vectors exchanged with two on-chip AllGathers (batches 0-7 / 8-15) so
    the first exchange overlaps the second half of the batch loop.

All heavy operands are f16 (host-cast): halves HBM traffic and runs the
PE at full rate (f32/f32r matmuls are ~3-4x slower on TRN2).

Per-core dataflow:
  scores  : DVE scalar_tensor_tensor (f16 in, f32 accum) against a gpsimd
            partition-broadcast of the hidden row
  softmax : one gpsimd partition_all_reduce (max) per PAIR of batches +
            ACT exp. NO normalization: the denominator sum(e) is produced
            by an extra ones-column matmul and divided out post-exchange.
  weighted: PE matmuls with column-masked f16 e-vectors (lhsT [s,8], col
            b%8 nonzero). PSUM out rows must start at partition 0/32/64/96
            so the 4096 d-cols are spread over 4 partition-base groups.
  linear  : hidden half from host-pretransposed hidT; weighted half from
            the allgathered, denominator-normalized vectors via PE
            transposes. Output batch rows are in exchange order
            (g,k,j) -> b = k*16+g*8+j; the final store un-permutes.
"""

import sys

if "/opt/trn_rl_repo" not in sys.path:
    sys.path.insert(0, "/opt/trn_rl_repo")

import numpy as np


# ----------------------------------------------------------------------------
# Program builder
# ----------------------------------------------------------------------------

def build_program(S=256, B=128, D=4096, n_cores=8):
    import concourse.bacc as bacc
    import concourse.bass_isa as bass_isa
    import concourse.mybir as mybir
    import concourse.tile as tile
    from concourse import library_config

    f32 = mybir.dt.float32
    f16 = mybir.dt.float16
    P = 128
    Bc = B // n_cores                 # batches per core (16)
    ST = S // P                       # s-tiles per batch (2)
    F = 2 * D                         # concat feature dim (8192)
    DOUT = D // n_cores               # output-dim shard per core (512)
    NKF = F // P                      # 128-wide k-chunks of the linear (64)
    ND = D // P                       # 128-wide d-chunks (32)
    G = Bc // 2                       # batch pairs (8)
    HB = Bc // 2                      # batches per exchange group (8)
    WCC = D + 1                       # exchange payload width (4096 d + den)

    nc = bacc.Bacc(None, target_bir_lowering=False)

    inp = nc.dram_tensor("inp", [Bc, ST, P, D], f16, kind="ExternalInput")
    hid = nc.dram_tensor("hid", [Bc, D], f16, kind="ExternalInput")
    hT = nc.dram_tensor("hT", [P, ND, P], f16, kind="ExternalInput")
    wt = nc.dram_tensor("wt", [P, NKF, DOUT], f16, kind="ExternalInput")
    bias = nc.dram_tensor("bias", [1, DOUT], f32, kind="ExternalInput")
    ident = nc.dram_tensor("ident", [P, P], f32, kind="ExternalInput")
    ones = nc.dram_tensor("ones", [P, 8], f16, kind="ExternalInput")
    out = nc.dram_tensor("out", [B, DOUT], f32, kind="ExternalOutput")

    cc_in = [nc.dram_tensor(f"cc_in{g}", [HB, WCC], f16) for g in range(2)]
    cc_out = [
        nc.dram_tensor(f"cc_out{g}", [n_cores * HB, WCC], f16, addr_space="Shared")
        for g in range(2)
    ]

    inp_r = inp.rearrange("b t p d -> b p t d")

    with tile.TileContext(nc) as tc:
        import contextlib

        with contextlib.ExitStack() as ctx:
            persist = ctx.enter_context(tc.tile_pool(name="persist", bufs=1))

            nc.gpsimd.load_library(library_config.attn)

            # ---- prefetches on the ACT (scalar) HWDGE queue ----
            ident_sb = persist.tile([P, P], f32)
            nc.scalar.dma_start(out=ident_sb, in_=ident[:, :])
            ones_sb = persist.tile([P, 8], f16)
            nc.scalar.dma_start(out=ones_sb, in_=ones[:, :])
            hT_sb = persist.tile([P, ND, P], f16)
            nc.scalar.dma_start(out=hT_sb, in_=hT[:, :, :])
            wt_sb = persist.tile([P, NKF, DOUT], f16)
            for q in range(4):
                nc.scalar.dma_start(
                    out=wt_sb[:, q * 16 : (q + 1) * 16, :],
                    in_=wt[:, q * 16 : (q + 1) * 16, :],
                )
            bias_sb = persist.tile([1, DOUT], f32)
            nc.scalar.dma_start(out=bias_sb, in_=bias[:, :])

            # masked e-vectors: [s, t, col] per 8-batch group; col j of slice
            # (t, j) holds batch (grp*8+j)'s e-values, everything else zero
            diag = persist.tile([P, ST, 8, 8], f16)
            nc.vector.memset(diag[:, :, :, :].bitcast(f32), 0.0)

            # unnormalized weighted sums + denominators, f16, evac dest
            ws = persist.tile([P, 2, 1536], f16)
            den_sb = persist.tile([P, 2], f16)

            hrow = persist.tile([1, 2 * D], f16)

            # PSUM: banks 0-5. Matmul outputs may only start at partition
            # base 0/32/64, so the 4096 d-cols are spread as 1536/1536/1024
            # over those bases. Per base: group A in cols 0:1536, group B in
            # cols 1536:3072 (base64: A 0:1024, B 1024:2048, denominators in
            # cols 2048 (A) and 2560 (B)).
            linp = ctx.enter_context(tc.tile_pool(name="lin", bufs=1, space="PSUM"))
            out_ps = linp.tile([P, DOUT], f32)
            # (base, out_col within group, d_lo); n=512 (psum bank limit)
            MM_CHUNKS = [
                (0, 0, 0),
                (0, 512, 512),
                (0, 1024, 1024),
                (32, 0, 1536),
                (32, 512, 2048),
                (32, 1024, 2560),
                (64, 0, 3072),
                (64, 512, 3584),
            ]
            B_OFF = {0: 1536, 32: 1536, 64: 1024}
            DEN_COL = (2048, 2560)
            # evac: (base, cc_col, width)
            EVAC = [(0, 0, 1536), (32, 1536, 1536), (64, 3072, 1024)]

            # loop pools live in their own stack, closed before the tail so
            # the tail tiles (wag etc.) reuse the loop's SBUF
            loop_stack = ctx.enter_context(contextlib.ExitStack())
            natp = loop_stack.enter_context(tc.tile_pool(name="nat", bufs=3))
            hbp = loop_stack.enter_context(tc.tile_pool(name="hb", bufs=2))
            prodp = loop_stack.enter_context(tc.tile_pool(name="prod", bufs=2))
            smalls = loop_stack.enter_context(tc.tile_pool(name="smalls", bufs=2))

            # last-opened so it can be closed (LIFO) right after the loop,
            # freeing its 6 PSUM banks for the tail's transpose pipeline
            wacc_stack = ctx.enter_context(contextlib.ExitStack())
            waccp = wacc_stack.enter_context(
                tc.tile_pool(name="wacc", bufs=1, space="PSUM")
            )
            wacc = waccp.tile([P, 3072], f32)

            hid_pairs = hid.rearrange("(g two) d -> g (two d)", two=2)

            def emit_hb(g):
                # stage hidden rows 2g,2g+1 then gpsimd-broadcast to all
                # partitions (gpsimd is otherwise idle in the loop)
                nc.sync.dma_start(out=hrow, in_=hid_pairs[g : g + 1, :])
                hb = hbp.tile([P, 2 * D], f16, tag="hb")
                nc.gpsimd.partition_broadcast(hb, hrow)
                return hb

            def evac_group(g):
                for base, cc_col, width in EVAC:
                    src0 = B_OFF[base] if g == 1 else 0
                    nc.scalar.activation(
                        out=ws[base : base + 8, g, 0:width],
                        in_=wacc[base : base + 8, src0 : src0 + width],
                        func=mybir.ActivationFunctionType.Copy,
                    )
                nc.scalar.activation(
                    out=den_sb[64:72, g : g + 1],
                    in_=wacc[64:72, DEN_COL[g] : DEN_COL[g] + 1],
                    func=mybir.ActivationFunctionType.Copy,
                )
                for base, cc_col, width in EVAC:
                    nc.scalar.dma_start(
                        out=cc_in[g][:, cc_col : cc_col + width],
                        in_=ws[base : base + 8, g, 0:width],
                    )
                nc.scalar.dma_start(
                    out=cc_in[g][:, D : D + 1], in_=den_sb[64:72, g : g + 1]
                )

            def emit_allgather(g):
                nc.gpsimd.collective_compute(
                    "AllGather",
                    mybir.AluOpType.bypass,
                    replica_groups=[list(range(n_cores))],
                    ins=[cc_in[g][:, :]],
                    outs=[cc_out[g][:, :]],
                )

            # ---------------- attention (batch loop) ----------------
            hbs = {0: emit_hb(0)}
            nats = {}
            e_tiles = {}
            sc2 = None

            for b in range(Bc):
                grp, j = divmod(b, 8)
                g2 = b // 2

                nat = natp.tile([P, ST, D], f16, tag="nat")
                nc.sync.dma_start(out=nat, in_=inp_r[b])
                nats[b] = nat

                hb = hbs[g2][:, (b % 2) * D : (b % 2 + 1) * D]
                if b % 2 == 0 and g2 + 1 < G:
                    hbs[g2 + 1] = emit_hb(g2 + 1)

                if b % 2 == 0:
                    sc2 = smalls.tile([P, 4], f32, tag="sc")
                for t in range(ST):
                    # 2-input tensor_tensor runs faster DVE fp16 modes (the
                    # 3-input scalar_tensor_tensor cannot); the free-dim sum
                    # is split between the ACT accumulator (t=0) and a DVE
                    # tensor_reduce (t=1) to balance the two engines.
                    prod = prodp.tile([P, D], f16, tag="prod")
                    nc.vector.tensor_tensor(
                        out=prod, in0=nat[:, t, :], in1=hb, op=mybir.AluOpType.mult
                    )
                    col = (b % 2) * 2 + t
                    if t == 0:
                        nc.scalar.activation(
                            out=prod,
                            in_=prod,
                            func=mybir.ActivationFunctionType.Copy,
                            accum_out=sc2[:, col : col + 1],
                        )
                    else:
                        nc.vector.tensor_reduce(
                            out=sc2[:, col : col + 1],
                            in_=prod,
                            axis=mybir.AxisListType.X,
                            op=mybir.AluOpType.add,
                        )

                if b % 2 == 1:
                    # one partition all-reduce (max) for the pair
                    mx4 = smalls.tile([P, 4], f32, tag="mx")
                    nc.gpsimd.partition_all_reduce(
                        mx4, sc2, channels=P, reduce_op=bass_isa.ReduceOp.max
                    )
                    for bb in (b - 1, b):
                        o = (bb % 2) * 2
                        negm = smalls.tile([P, 1], f32, tag=f"negm{bb % 2}")
                        nc.vector.tensor_reduce(
                            out=negm, in_=mx4[:, o : o + 2], axis=mybir.AxisListType.X,
                            op=mybir.AluOpType.max, negate=True,
                        )
                        e_b = smalls.tile([P, ST], f16, tag=f"e{bb % 2}")
                        nc.scalar.activation(
                            out=e_b,
                            in_=sc2[:, o : o + 2],
                            func=mybir.ActivationFunctionType.Exp,
                            bias=negm,
                            scale=1.0,
                        )
                        e_tiles[bb] = e_b

                    # weighted-sum matmuls for both batches of the pair
                    for bb in (b - 1, b):
                        gg, jj = divmod(bb, 8)
                        e_b = e_tiles.pop(bb)
                        natb = nats.pop(bb)
                        for t in range(ST):
                            nc.scalar.activation(
                                out=diag[:, t, jj, jj : jj + 1],
                                in_=e_b[:, t : t + 1],
                                func=mybir.ActivationFunctionType.Copy,
                            )
                        for t in range(ST):
                            lhsT = diag[:, t, jj, :]
                            st = jj == 0 and t == 0
                            sp = jj == 7 and t == ST - 1
                            for base, col, d_lo in MM_CHUNKS:
                                co = col + (B_OFF[base] if gg == 1 else 0)
                                nc.tensor.matmul(
                                    wacc[base : base + 8, co : co + 512],
                                    lhsT,
                                    natb[:, t, d_lo : d_lo + 512],
                                    start=st,
                                    stop=sp,
                                )
                            nc.tensor.matmul(
                                wacc[64:72, DEN_COL[gg] : DEN_COL[gg] + 1],
                                lhsT,
                                ones_sb[:, 0:1],
                                start=st,
                                stop=sp,
                            )

                    # hidden half of the linear, spread through the loop
                    for i in (2 * (b - 1), 2 * b - 1, 2 * b, 2 * b + 1):
                        nc.tensor.matmul(
                            out_ps,
                            hT_sb[:, i, :],
                            wt_sb[:, ND + i, :],
                            start=(i == 0),
                            stop=False,
                            skip_group_check=True,
                        )

                    # evac A two pairs after its matmuls stop so the ACT
                    # queue never head-of-line blocks the score pipeline
                    if b == 9:
                        evac_group(0)
                    if b == 11:
                        emit_allgather(0)
                    if b == Bc - 1:
                        evac_group(1)
                        emit_allgather(1)
                        wacc_stack.close()
                        loop_stack.close()

            # ---------------- linear tail (weighted half) ----------------
            with contextlib.ExitStack() as lin_ctx:
                tailp = lin_ctx.enter_context(tc.tile_pool(name="tail", bufs=1))
                wnp = lin_ctx.enter_context(tc.tile_pool(name="wn", bufs=2))
                wTp = lin_ctx.enter_context(tc.tile_pool(name="wT", bufs=2))
                tpp = lin_ctx.enter_context(
                    tc.tile_pool(name="tp", bufs=3, space="PSUM")
                )

                wag = tailp.tile([P, WCC], f16)
                recip = tailp.tile([P, 1], f32)

                # per exchange group (rows g*64:(g+1)*64 of the permuted B)
                for g in range(2):
                    r0 = g * 64
                    idg = ident_sb[r0 : r0 + 64, r0 : r0 + 64]
                    nc.sync.dma_start(
                        out=wag[r0 : r0 + 64, :], in_=cc_out[g][:, :]
                    )
                    nc.vector.reciprocal(
                        recip[r0 : r0 + 64], wag[r0 : r0 + 64, D : D + 1]
                    )
                    tp = None
                    for f in range(8):  # normalize 512 d-cols per fill
                        wn = wnp.tile([P, 512], f32, tag="wn")
                        nc.scalar.activation(
                            out=wn[r0 : r0 + 64, :],
                            in_=wag[r0 : r0 + 64, f * 512 : (f + 1) * 512],
                            func=mybir.ActivationFunctionType.Copy,
                            scale=recip[r0 : r0 + 64],
                        )
                        if f % 2 == 0:
                            tp = tpp.tile([P, DOUT], f32, tag="tp")
                        for q in range(4):
                            nc.tensor.transpose(
                                tp[:, (f % 2) * 256 + q * 64 : (f % 2) * 256 + (q + 1) * 64],
                                wn[r0 : r0 + 64, q * P : (q + 1) * P],
                                idg,
                            )
                        if f % 2 == 1:
                            wT = wTp.tile([P, DOUT], f16, tag="wT")
                            nc.vector.tensor_copy(wT, tp)
                            for q in range(8):
                                c = (f - 1) * 4 + q
                                nc.tensor.matmul(
                                    out_ps[r0 : r0 + 64, :],
                                    wT[:, q * 64 : (q + 1) * 64],
                                    wt_sb[:, c, :],
                                    start=False,
                                    stop=(c == ND - 1),
                                    skip_group_check=True,
                                )

                # bias add + store (un-permute exchange order back to b)
                bias_bc = tailp.tile([P, DOUT], f32)
                nc.gpsimd.partition_broadcast(bias_bc, bias_sb)
                # rows are in exchange order (g,k,j); host un-permutes
                out_sb = tailp.tile([P, DOUT], f32)
                nc.vector.tensor_add(out_sb, out_ps, bias_bc)
                nc.sync.dma_start(out=out[:, :], in_=out_sb)

    nc.finalize()
    return nc


_CACHE = {}


def _get_program(S, B, D, n_cores):
    key = (S, B, D, n_cores)
    if key not in _CACHE:
        _CACHE[key] = build_program(S, B, D, n_cores)
    return _CACHE[key]


def make_in_maps(inp, hidden, W, b, n_cores=8):
    """Shard host inputs into per-core input maps (f16 for heavy operands)."""
    f16 = np.float16
    S, B, D = inp.shape
    Bc = B // n_cores
    DOUT = W.shape[0] // n_cores
    P = 128

    # batch permutation of the exchange order: i=(g,k,j) -> b = k*16+g*8+j
    perm = [k * Bc + g * 8 + j for g in range(2) for k in range(n_cores) for j in range(8)]
    hTg = np.ascontiguousarray(hidden[0].T.astype(f16))          # [D, B]
    hT_pi = hTg[:, perm]                                          # [D, B]
    hT_pack = np.ascontiguousarray(
        hT_pi.reshape(D // P, P, B).transpose(1, 0, 2)
    )                                                             # [P, ND, B]

    ident = np.eye(P, dtype=np.float32)
    ones = np.ones((P, 8), dtype=f16)

    in_maps = []
    for k in range(n_cores):
        inp_k = inp[:, k * Bc : (k + 1) * Bc, :]                  # [S, Bc, D]
        inp_pack = np.ascontiguousarray(
            inp_k.transpose(1, 0, 2).reshape(Bc, 2, P, D).astype(f16)
        )
        wtk = W[k * DOUT : (k + 1) * DOUT, :].T                   # [F, DOUT]
        wt_pack = np.ascontiguousarray(
            wtk.reshape(2 * D // P, P, DOUT).transpose(1, 0, 2).astype(f16)
        )                                                         # [P, NKF, DOUT]
        in_maps.append(
            {
                "inp": inp_pack,
                "hid": np.ascontiguousarray(
                    hidden[0, k * Bc : (k + 1) * Bc, :].astype(f16)
                ),
                "hT": hT_pack,
                "wt": wt_pack,
                "bias": np.ascontiguousarray(
                    b[k * DOUT : (k + 1) * DOUT].reshape(1, DOUT).astype(np.float32)
                ),
                "ident": ident,
                "ones": ones,
            }
        )
    return in_maps


def kernel(inp, hidden, W, b, trace=False):
    from concourse.bass_utils import run_bass_kernel_spmd

    inp = np.asarray(inp, dtype=np.float32)
    hidden = np.asarray(hidden, dtype=np.float32)
    W = np.asarray(W, dtype=np.float32)
    b = np.asarray(b, dtype=np.float32)

    S, B, D = inp.shape
    n_cores = 8
    nc = _get_program(S, B, D, n_cores)
    in_maps = make_in_maps(inp, hidden, W, b, n_cores)
    res = run_bass_kernel_spmd(nc, in_maps, core_ids=list(range(n_cores)))
    # per-core out rows are in exchange order i=(g,k,j) <-> b=k*16+g*8+j
    outs = [
        np.asarray(res.results[k]["out"])
        .reshape(2, n_cores, 8, -1)
        .transpose(1, 0, 2, 3)
        .reshape(B, -1)
        for k in range(n_cores)
    ]
    full = np.concatenate(outs, axis=1)  # [B, D]
    if trace:
        return full[None, :, :], res
    return full[None, :, :]


# revision 43
# speedup vs baseline: 1.8809x; 1.8809x over previous
"""Trainium2 Bass kernel for rank-1 attention + linear (nn_Attention).

Reference computation (S=256, B=128, D=4096):
    scores   = einsum('sbd,bd->bs', inp, hidden[0])      # dot each enc state with hidden
    attn     = softmax(scores, axis=1)                   # over S
    weighted = einsum('bs,sbd->bd', attn, inp)
    concat   = [weighted, hidden[0]]   # [B, 2D]
    out      = concat @ W.T + b        # [1, B, D]

Distribution over 8 NeuronCores:
  - attention part: data-parallel over B (16 batches per core)
  - linear part: W sharded over output dim (512 rows per core); weighted
    vectors exchanged with two on-chip AllGathers (batches 0-7 / 8-15) so
    the first exchange overlaps the second half of the batch loop.

All heavy operands are f16 (host-cast): halves HBM traffic and runs the
PE at full rate (f32/f32r matmuls are ~3-4x slower on TRN2).

Per-core dataflow:
  scores  : DVE scalar_tensor_tensor (f16 in, f32 accum) against a gpsimd
            partition-broadcast of the hidden row
  softmax : one gpsimd partition_all_reduce (max) per PAIR of batches +
            ACT exp. NO normalization: the denominator sum(e) is produced
            by an extra ones-column matmul and divided out post-exchange.
  weighted: PE matmuls with column-masked f16 e-vectors (lhsT [s,8], col
            b%8 nonzero). PSUM out rows must start at partition 0/32/64/96
            so the 4096 d-cols are spread over 4 partition-base groups.
  linear  : hidden half from host-pretransposed hidT; weighted half from
            the allgathered, denominator-normalized vectors via PE
            transposes. Output batch rows are in exchange order
            (g,k,j) -> b = k*16+g*8+j; the final store un-permutes.
"""

import sys

if "/opt/trn_rl_repo" not in sys.path:
    sys.path.insert(0, "/opt/trn_rl_repo")

import numpy as np


# ----------------------------------------------------------------------------
# Program builder
# ----------------------------------------------------------------------------

def build_program(S=256, B=128, D=4096, n_cores=8):
    import concourse.bacc as bacc
    import concourse.bass_isa as bass_isa
    import concourse.mybir as mybir
    import concourse.tile as tile
    from concourse import library_config

    f32 = mybir.dt.float32
    f16 = mybir.dt.float16
    P = 128
    Bc = B // n_cores                 # batches per core (16)
    ST = S // P                       # s-tiles per batch (2)
    F = 2 * D                         # concat feature dim (8192)
    DOUT = D // n_cores               # output-dim shard per core (512)
    NKF = F // P                      # 128-wide k-chunks of the linear (64)
    ND = D // P                       # 128-wide d-chunks (32)
    G = Bc // 2                       # batch pairs (8)
    HB = Bc // 2                      # batches per exchange group (8)
    WCC = D + 1                       # exchange payload width (4096 d + den)

    nc = bacc.Bacc(None, target_bir_lowering=False)

    inp = nc.dram_tensor("inp", [Bc, ST, P, D], f16, kind="ExternalInput")
    hid = nc.dram_tensor("hid", [Bc, D], f16, kind="ExternalInput")
    hT = nc.dram_tensor("hT", [P, ND, P], f16, kind="ExternalInput")
    wt = nc.dram_tensor("wt", [P, NKF, DOUT], f16, kind="ExternalInput")
    bias = nc.dram_tensor("bias", [1, DOUT], f32, kind="ExternalInput")
    ident = nc.dram_tensor("ident", [P, P], f32, kind="ExternalInput")
    ones = nc.dram_tensor("ones", [P, 8], f16, kind="ExternalInput")
    identf16 = nc.dram_tensor("identf16", [8, 8], f16, kind="ExternalInput")
    out = nc.dram_tensor("out", [B, DOUT], f32, kind="ExternalOutput")

    cc_in = [nc.dram_tensor(f"cc_in{g}", [HB, WCC], f16) for g in range(2)]
    cc_out = [
        nc.dram_tensor(f"cc_out{g}", [n_cores * HB, WCC], f16, addr_space="Shared")
        for g in range(2)
    ]

    inp_r = inp.rearrange("b t p d -> b p t d")

    with tile.TileContext(nc) as tc:
        import contextlib

        with contextlib.ExitStack() as ctx:
            persist = ctx.enter_context(tc.tile_pool(name="persist", bufs=1))

            nc.gpsimd.load_library(library_config.attn)

            # ---- prefetches on the ACT (scalar) HWDGE queue ----
            ident_sb = persist.tile([P, P], f32)
            nc.scalar.dma_start(out=ident_sb, in_=ident[:, :])
            ones_sb = persist.tile([P, 8], f16)
            nc.scalar.dma_start(out=ones_sb, in_=ones[:, :])
            idf_sb = persist.tile([8, 8], f16)
            nc.scalar.dma_start(out=idf_sb, in_=identf16[:, :])
            hT_sb = persist.tile([P, ND, P], f16)
            nc.scalar.dma_start(out=hT_sb, in_=hT[:, :, :])
            # wt streams during the loop (hidden half first) so the head
            # isn't prefetch-bound; see the b in (0,2,8,10) emissions below
            wt_sb = persist.tile([P, NKF, DOUT], f16)
            bias_sb = persist.tile([1, DOUT], f32)
            nc.scalar.dma_start(out=bias_sb, in_=bias[:, :])

            # masked e-vectors: [s, t, col] per 8-batch group; col j of slice
            # (t, j) holds batch (grp*8+j)'s e-values, everything else zero
            diag = persist.tile([P, ST, 8, 8], f16)
            nc.vector.memset(diag[:, :, :, :].bitcast(f32), 0.0)

            # unnormalized weighted sums + denominators, f16, evac dest
            ws = persist.tile([P, 2, 1536], f16)
            den_sb = persist.tile([P, 2], f16)

            # PSUM: banks 0-5. Matmul outputs may only start at partition
            # base 0/32/64, so the 4096 d-cols are spread as 1536/1536/1024
            # over those bases. Per base: group A in cols 0:1536, group B in
            # cols 1536:3072 (base64: A 0:1024, B 1024:2048, denominators in
            # cols 2048 (A) and 2560 (B)).
            # 2 banks: linear accumulator (cols 0:512) + softmax-transpose
            # scratch (scT2 rows 0:2 cols 512:768, e-columns 768:770)
            linp = ctx.enter_context(tc.tile_pool(name="lin", bufs=1, space="PSUM"))
            lin_ps = linp.tile([P, 1024], f32)
            out_ps = lin_ps[:, 0:DOUT]
            scT2 = lin_ps[0:2, DOUT : DOUT + 2 * P]
            ebs = [
                lin_ps[:, DOUT + 2 * P + t : DOUT + 2 * P + t + 1].bitcast(f16)
                for t in range(ST)
            ]
            # (base, out_col within group, d_lo); n=512 (psum bank limit)
            MM_CHUNKS = [
                (0, 0, 0),
                (0, 512, 512),
                (0, 1024, 1024),
                (32, 0, 1536),
                (32, 512, 2048),
                (32, 1024, 2560),
                (64, 0, 3072),
                (64, 512, 3584),
            ]
            B_OFF = {0: 1536, 32: 1536, 64: 1024}
            DEN_COL = (2048, 2560)
            # evac: (base, cc_col, width)
            EVAC = [(0, 0, 1536), (32, 1536, 1536), (64, 3072, 1024)]

            # loop pools live in their own stack, closed before the tail so
            # the tail tiles (wag etc.) reuse the loop's SBUF
            loop_stack = ctx.enter_context(contextlib.ExitStack())
            natp = loop_stack.enter_context(tc.tile_pool(name="nat", bufs=4))
            hbp = loop_stack.enter_context(tc.tile_pool(name="hb", bufs=2))
            prodp = loop_stack.enter_context(tc.tile_pool(name="prod", bufs=2))
            smalls = loop_stack.enter_context(tc.tile_pool(name="smalls", bufs=2))

            # last-opened so it can be closed (LIFO) right after the loop,
            # freeing its 6 PSUM banks for the tail's transpose pipeline
            wacc_stack = ctx.enter_context(contextlib.ExitStack())
            waccp = wacc_stack.enter_context(
                tc.tile_pool(name="wacc", bufs=1, space="PSUM")
            )
            wacc = waccp.tile([P, 3072], f32)

            hid_pairs = hid.rearrange("(g two) d -> g (two d)", two=2)

            def emit_hb(g):
                # broadcast hidden rows 2g,2g+1 to all partitions straight
                # from HBM: 0-stride partition dim -> 128 same-row descriptors.
                # (gpsimd partition_broadcast stalls the DVE via SBUF-port
                # contention - measured 2.25us -> 12us tensor_tensor.)
                hb = hbp.tile([P, 2 * D], f16, tag="hb")
                nc.sync.dma_start(
                    out=hb, in_=hid_pairs[g : g + 1, :].partition_broadcast(P)
                )
                return hb

            def evac_group(g):
                for base, cc_col, width in EVAC:
                    src0 = B_OFF[base] if g == 1 else 0
                    nc.scalar.activation(
                        out=ws[base : base + 8, g, 0:width],
                        in_=wacc[base : base + 8, src0 : src0 + width],
                        func=mybir.ActivationFunctionType.Copy,
                    )
                nc.scalar.activation(
                    out=den_sb[64:72, g : g + 1],
                    in_=wacc[64:72, DEN_COL[g] : DEN_COL[g] + 1],
                    func=mybir.ActivationFunctionType.Copy,
                )
                for base, cc_col, width in EVAC:
                    nc.scalar.dma_start(
                        out=cc_in[g][:, cc_col : cc_col + width],
                        in_=ws[base : base + 8, g, 0:width],
                    )
                nc.scalar.dma_start(
                    out=cc_in[g][:, D : D + 1], in_=den_sb[64:72, g : g + 1]
                )

            def emit_allgather(g):
                nc.gpsimd.collective_compute(
                    "AllGather",
                    mybir.AluOpType.bypass,
                    replica_groups=[list(range(n_cores))],
                    ins=[cc_in[g][:, :]],
                    outs=[cc_out[g][:, :]],
                )

            # ---------------- attention (batch loop) ----------------
            hbs = {0: emit_hb(0)}
            nats = {}
            e_tiles = {}
            sc2 = None

            for b in range(Bc):
                grp, j = divmod(b, 8)
                g2 = b // 2

                nat = natp.tile([P, ST, D], f16, tag="nat")
                nc.sync.dma_start(out=nat, in_=inp_r[b])
                nats[b] = nat

                # stream wt in 2MB chunks: hidden half early, weighted late
                if b in (0, 2, 8, 10):
                    q = {0: 2, 2: 3, 8: 0, 10: 1}[b]
                    nc.scalar.dma_start(
                        out=wt_sb[:, q * 16 : (q + 1) * 16, :],
                        in_=wt[:, q * 16 : (q + 1) * 16, :],
                    )

                hb = hbs[g2][:, (b % 2) * D : (b % 2 + 1) * D]
                if b % 2 == 0 and g2 + 1 < G:
                    hbs[g2 + 1] = emit_hb(g2 + 1)

                if b % 2 == 0:
                    sc2 = smalls.tile([P, 4], f32, tag="sc")
                for t in range(ST):
                    # 2-input tensor_tensor runs faster DVE fp16 modes (the
                    # 3-input scalar_tensor_tensor cannot); the free-dim sum
                    # is split between the ACT accumulator (t=0) and a DVE
                    # tensor_reduce (t=1) to balance the two engines.
                    prod = prodp.tile([P, D], f16, tag="prod")
                    nc.vector.tensor_tensor(
                        out=prod, in0=nat[:, t, :], in1=hb, op=mybir.AluOpType.mult
                    )
                    col = (b % 2) * 2 + t
                    if t == 0:
                        nc.scalar.activation(
                            out=prod,
                            in_=prod,
                            func=mybir.ActivationFunctionType.Copy,
                            accum_out=sc2[:, col : col + 1],
                        )
                    else:
                        nc.vector.tensor_reduce(
                            out=sc2[:, col : col + 1],
                            in_=prod,
                            axis=mybir.AxisListType.X,
                            op=mybir.AluOpType.add,
                        )

                if b % 2 == 1:
                    # pair softmax via PE transposes (no gpsimd): scT2 row r
                    # = batch b-1+r, cols (t,s); per-batch max on DVE (PSUM
                    # read); exp on ACT; transpose back for the masked cols
                    nc.tensor.transpose(scT2[:, 0:P], sc2[:, 0:4:2], ident_sb)
                    nc.tensor.transpose(scT2[:, P : 2 * P], sc2[:, 1:4:2], ident_sb)
                    negm2 = smalls.tile([2, 1], f32, tag="negm")
                    nc.vector.tensor_reduce(
                        out=negm2, in_=scT2, axis=mybir.AxisListType.X,
                        op=mybir.AluOpType.max, negate=True,
                    )
                    eT2 = smalls.tile([2, 2 * P], f16, tag="eT")
                    nc.scalar.activation(
                        out=eT2,
                        in_=scT2,
                        func=mybir.ActivationFunctionType.Exp,
                        bias=negm2,
                        scale=1.0,
                    )
                    for t in range(ST):
                        nc.tensor.transpose(
                            ebs[t], eT2[:, t * P : (t + 1) * P], idf_sb[0:2, 0:2]
                        )

                    # weighted-sum matmuls for both batches of the pair
                    for bb in (b - 1, b):
                        gg, jj = divmod(bb, 8)
                        r = bb - (b - 1)
                        natb = nats.pop(bb)
                        for t in range(ST):
                            nc.scalar.activation(
                                out=diag[:, t, jj, jj : jj + 1],
                                in_=ebs[t][:, r : r + 1],
                                func=mybir.ActivationFunctionType.Copy,
                            )
                        for t in range(ST):
                            lhsT = diag[:, t, jj, :]
                            st = jj == 0 and t == 0
                            sp = jj == 7 and t == ST - 1
                            for base, col, d_lo in MM_CHUNKS:
                                co = col + (B_OFF[base] if gg == 1 else 0)
                                nc.tensor.matmul(
                                    wacc[base : base + 8, co : co + 512],
                                    lhsT,
                                    natb[:, t, d_lo : d_lo + 512],
                                    start=st,
                                    stop=sp,
                                )
                            nc.tensor.matmul(
                                wacc[64:72, DEN_COL[gg] : DEN_COL[gg] + 1],
                                lhsT,
                                ones_sb[:, 0:1],
                                start=st,
                                stop=sp,
                            )

                    # hidden half of the linear, spread over pairs 1..7
                    # (pair 0's slot would race the first wt chunk's DMA)
                    p2 = b // 2
                    if p2 >= 1:
                        for i in range(4 * (p2 - 1), 4 * (p2 - 1) + 4):
                            nc.tensor.matmul(
                                out_ps,
                                hT_sb[:, i, :],
                                wt_sb[:, ND + i, :],
                                start=(i == 0),
                                stop=False,
                                skip_group_check=True,
                            )

                    # evac A two pairs after its matmuls stop so the ACT
                    # queue never head-of-line blocks the score pipeline
                    if b == 9:
                        evac_group(0)
                    if b == 11:
                        emit_allgather(0)
                    if b == Bc - 1:
                        evac_group(1)
                        emit_allgather(1)
                        wacc_stack.close()
                        loop_stack.close()

            # ---------------- linear tail (weighted half) ----------------
            with contextlib.ExitStack() as lin_ctx:
                tailp = lin_ctx.enter_context(tc.tile_pool(name="tail", bufs=1))
                wnp = lin_ctx.enter_context(tc.tile_pool(name="wn", bufs=2))
                wTp = lin_ctx.enter_context(tc.tile_pool(name="wT", bufs=2))
                tpp = lin_ctx.enter_context(
                    tc.tile_pool(name="tp", bufs=3, space="PSUM")
                )

                wag = tailp.tile([P, WCC], f16)
                recip = tailp.tile([P, 1], f32)

                # last 4 hidden-half k-chunks (pairs only cover 0..27)
                for i in range(28, 32):
                    nc.tensor.matmul(
                        out_ps,
                        hT_sb[:, i, :],
                        wt_sb[:, ND + i, :],
                        start=False,
                        stop=False,
                        skip_group_check=True,
                    )

                # per exchange group (rows g*64:(g+1)*64 of the permuted B)
                for g in range(2):
                    r0 = g * 64
                    idg = ident_sb[r0 : r0 + 64, r0 : r0 + 64]
                    nc.sync.dma_start(
                        out=wag[r0 : r0 + 64, :], in_=cc_out[g][:, :]
                    )
                    nc.vector.reciprocal(
                        recip[r0 : r0 + 64], wag[r0 : r0 + 64, D : D + 1]
                    )
                    tp = None
                    for f in range(8):  # normalize 512 d-cols per fill
                        wn = wnp.tile([P, 512], f32, tag="wn")
                        nc.scalar.activation(
                            out=wn[r0 : r0 + 64, :],
                            in_=wag[r0 : r0 + 64, f * 512 : (f + 1) * 512],
                            func=mybir.ActivationFunctionType.Copy,
                            scale=recip[r0 : r0 + 64],
                        )
                        if f % 2 == 0:
                            tp = tpp.tile([P, DOUT], f32, tag="tp")
                        for q in range(4):
                            nc.tensor.transpose(
                                tp[:, (f % 2) * 256 + q * 64 : (f % 2) * 256 + (q + 1) * 64],
                                wn[r0 : r0 + 64, q * P : (q + 1) * P],
                                idg,
                            )
                        if f % 2 == 1:
                            wT = wTp.tile([P, DOUT], f16, tag="wT")
                            nc.vector.tensor_copy(wT, tp)
                            for q in range(8):
                                c = (f - 1) * 4 + q
                                nc.tensor.matmul(
                                    out_ps[r0 : r0 + 64, :],
                                    wT[:, q * 64 : (q + 1) * 64],
                                    wt_sb[:, c, :],
                                    start=False,
                                    stop=(c == ND - 1),
                                    skip_group_check=True,
                                )

                # bias add + store (un-permute exchange order back to b)
                bias_bc = tailp.tile([P, DOUT], f32)
                nc.gpsimd.partition_broadcast(bias_bc, bias_sb)
                # rows are in exchange order (g,k,j); host un-permutes
                out_sb = tailp.tile([P, DOUT], f32)
                nc.vector.tensor_add(out_sb, out_ps, bias_bc)
                nc.sync.dma_start(out=out[:, :], in_=out_sb)

    nc.finalize()
    return nc


_CACHE = {}


def _get_program(S, B, D, n_cores):
    key = (S, B, D, n_cores)
    if key not in _CACHE:
        _CACHE[key] = build_program(S, B, D, n_cores)
    return _CACHE[key]


def make_in_maps(inp, hidden, W, b, n_cores=8):
    """Shard host inputs into per-core input maps (f16 for heavy operands)."""
    f16 = np.float16
    S, B, D = inp.shape
    Bc = B // n_cores
    DOUT = W.shape[0] // n_cores
    P = 128

    # batch permutation of the exchange order: i=(g,k,j) -> b = k*16+g*8+j
    perm = [k * Bc + g * 8 + j for g in range(2) for k in range(n_cores) for j in range(8)]
    hTg = np.ascontiguousarray(hidden[0].T.astype(f16))          # [D, B]
    hT_pi = hTg[:, perm]                                          # [D, B]
    hT_pack = np.ascontiguousarray(
        hT_pi.reshape(D // P, P, B).transpose(1, 0, 2)
    )                                                             # [P, ND, B]

    ident = np.eye(P, dtype=np.float32)
    ones = np.ones((P, 8), dtype=f16)

    in_maps = []
    for k in range(n_cores):
        inp_k = inp[:, k * Bc : (k + 1) * Bc, :]                  # [S, Bc, D]
        inp_pack = np.ascontiguousarray(
            inp_k.transpose(1, 0, 2).reshape(Bc, 2, P, D).astype(f16)
        )
        wtk = W[k * DOUT : (k + 1) * DOUT, :].T                   # [F, DOUT]
        wt_pack = np.ascontiguousarray(
            wtk.reshape(2 * D // P, P, DOUT).transpose(1, 0, 2).astype(f16)
        )                                                         # [P, NKF, DOUT]
        in_maps.append(
            {
                "inp": inp_pack,
                "hid": np.ascontiguousarray(
                    hidden[0, k * Bc : (k + 1) * Bc, :].astype(f16)
                ),
                "hT": hT_pack,
                "wt": wt_pack,
                "bias": np.ascontiguousarray(
                    b[k * DOUT : (k + 1) * DOUT].reshape(1, DOUT).astype(np.float32)
                ),
                "ident": ident,
                "ones": ones,
                "identf16": np.eye(8, dtype=f16),
            }
        )
    return in_maps


def kernel(inp, hidden, W, b, trace=False):
    from concourse.bass_utils import run_bass_kernel_spmd

    inp = np.asarray(inp, dtype=np.float32)
    hidden = np.asarray(hidden, dtype=np.float32)
    W = np.asarray(W, dtype=np.float32)
    b = np.asarray(b, dtype=np.float32)

    S, B, D = inp.shape
    n_cores = 8
    nc = _get_program(S, B, D, n_cores)
    in_maps = make_in_maps(inp, hidden, W, b, n_cores)
    res = run_bass_kernel_spmd(nc, in_maps, core_ids=list(range(n_cores)))
    # per-core out rows are in exchange order i=(g,k,j) <-> b=k*16+g*8+j
    outs = [
        np.asarray(res.results[k]["out"])
        .reshape(2, n_cores, 8, -1)
        .transpose(1, 0, 2, 3)
        .reshape(B, -1)
        for k in range(n_cores)
    ]
    full = np.concatenate(outs, axis=1)  # [B, D]
    if trace:
        return full[None, :, :], res
    return full[None, :, :]
